# revision 1
# baseline (speedup 1.0000x reference)
"""Trainium2 Bass kernel for EnhancedGraphSAGE (embed -> 2x SAGE-mean -> GAT -> MLP).

Self-contained: takes full inputs, shards node-wise across 8 NeuronCores
internally, returns the full [N, C] output.

Design:
- Nodes are relabeled by a random permutation into NID = 8*56*128 internal ids
  (core-major, then 128-dst "blocks"). Each core owns its 56 blocks' dsts.
- Edges grouped by dst block; within a block, split by src table half (int16
  index range of dma_gather), each half padded to 1024 slots (8 tiles of 128).
- Neighbor aggregation: dma_gather of 256B bf16 table rows (messages land
  [128 edge-slots, 16, 128]) followed by TensorE matmuls with host-prebuilt
  fp8 one-hot masks (lhsT = mask [128 edges, 128 dsts]) accumulating in PSUM.
- GAT: softmax computed without max-subtraction (exp of leaky_relu bounded);
  er[dst] broadcast to edges via maskT matmul; per-head ex weighting on DVE;
  Wg folded into W1 on the host (U = Wg_h @ W1_h), bg folded into b1.
- Two AllGathers only (post-SAGE1 / post-SAGE2 node features); the embed
  layer is computed fully replicated on every core.
"""

import numpy as np

import concourse.bacc as bacc
import concourse.bass as bass
import concourse.mybir as mybir
import concourse.tile as tile
from concourse.bass_utils import run_bass_kernel_spmd
from concourse.masks import make_identity

# Problem constants (hardcoded per spec)
N, E, IN, H, HEADS, C = 50000, 800000, 128, 64, 4, 40
SLOPE = 0.2

# Sharding geometry
NCORES = 8
NBLK = 56              # dst blocks per core
PB = 128               # dst slots per block
TPH = 8                # gather tiles per half (1024 idx limit of dma_gather)
TPB = 2 * TPH          # tiles per block
SLOTH = TPH * 128      # slots per half
S16 = SLOTH // 16      # idx columns in packed [128, S16] layout
OWN = NBLK * PB        # own nodes per core (7168)
NID = NCORES * OWN     # internal id space (57344)
HALFR = NID // 2       # table half split (28672 < 32768)
D = 128                # table row width (bf16 -> 256B rows)
CH = 512               # dense chunk (nodes per matmul)
NCH_OWN = OWN // CH    # 14
NCH_ALL = NID // CH    # 112

F32 = mybir.dt.float32
BF16 = mybir.dt.bfloat16
FP8 = mybir.dt.float8e4
I16 = mybir.dt.int16
NP_BF16 = mybir.dt.np(BF16)
NP_FP8 = mybir.dt.np(FP8)

_cached = {}


def _build_bass(upto=99):
    nc = bacc.Bacc("TRN2", target_bir_lowering=False, debug=False,
                   num_devices=NCORES)

    # ---- I/O ----
    xT = nc.dram_tensor("xT", [IN, NID], F32, kind="ExternalInput")
    xo = nc.dram_tensor("xo", [IN, OWN], F32, kind="ExternalInput")
    idx_in = nc.dram_tensor("idx_in", [NBLK, 128, 2, S16], I16, kind="ExternalInput")
    mask_in = nc.dram_tensor("mask_in", [NBLK, 128, TPB * 128], FP8, kind="ExternalInput")
    maskT_in = nc.dram_tensor("maskT_in", [NBLK, 128, TPB * 128], FP8, kind="ExternalInput")
    dgi_in = nc.dram_tensor("dgi_in", [NBLK, 128, 1], F32, kind="ExternalInput")

    wemb = nc.dram_tensor("wemb", [IN, H], F32, kind="ExternalInput")
    bembr = nc.dram_tensor("bembr", [1, H], F32, kind="ExternalInput")
    bembc = nc.dram_tensor("bembc", [H, 1], F32, kind="ExternalInput")
    ws1 = nc.dram_tensor("ws1", [H, H], F32, kind="ExternalInput")
    wn1 = nc.dram_tensor("wn1", [H, H], F32, kind="ExternalInput")
    bn1 = nc.dram_tensor("bn1", [H, 1], F32, kind="ExternalInput")
    ws2 = nc.dram_tensor("ws2", [H, H], F32, kind="ExternalInput")
    wn2 = nc.dram_tensor("wn2", [H, H], F32, kind="ExternalInput")
    bn2 = nc.dram_tensor("bn2", [H, 1], F32, kind="ExternalInput")
    wl_in = nc.dram_tensor("wl_in", [H, HEADS], F32, kind="ExternalInput")
    wr_in = nc.dram_tensor("wr_in", [H, HEADS], F32, kind="ExternalInput")
    ulo_in = nc.dram_tensor("ulo_in", [128, H], BF16, kind="ExternalInput")
    uhi_in = nc.dram_tensor("uhi_in", [128, H], BF16, kind="ExternalInput")
    b1p = nc.dram_tensor("b1p", [H, 1], F32, kind="ExternalInput")
    w2_in = nc.dram_tensor("w2_in", [H, C], F32, kind="ExternalInput")
    b2c = nc.dram_tensor("b2c", [C, 1], F32, kind="ExternalInput")

    out = nc.dram_tensor("out", [OWN, C], F32, kind="ExternalOutput")

    with tile.TileContext(nc) as tc:
        with (
            tc.tile_pool(name="wpool", bufs=1) as wp,
            tc.tile_pool(name="sbuf", bufs=3) as sb,
            tc.tile_pool(name="big", bufs=1) as bigp,
            tc.tile_pool(name="psum", bufs=2, space="PSUM") as pp,
            tc.tile_pool(name="dram", bufs=1, space="DRAM") as dram,
        ):
            # ---- constants / weights resident in SBUF ----
            w_emb = wp.tile([IN, H], F32)
            nc.sync.dma_start(w_emb[:], wemb[:])
            b_embr = wp.tile([1, H], F32)
            nc.sync.dma_start(b_embr[:], bembr[:])
            b_embc = wp.tile([H, 1], F32)
            nc.sync.dma_start(b_embc[:], bembc[:])
            w_s1 = wp.tile([H, H], F32); nc.sync.dma_start(w_s1[:], ws1[:])
            w_n1 = wp.tile([H, H], F32); nc.sync.dma_start(w_n1[:], wn1[:])
            b_n1 = wp.tile([H, 1], F32); nc.sync.dma_start(b_n1[:], bn1[:])
            w_s2 = wp.tile([H, H], F32); nc.sync.dma_start(w_s2[:], ws2[:])
            w_n2 = wp.tile([H, H], F32); nc.sync.dma_start(w_n2[:], wn2[:])
            b_n2 = wp.tile([H, 1], F32); nc.sync.dma_start(b_n2[:], bn2[:])
            w_l = wp.tile([H, HEADS], F32); nc.sync.dma_start(w_l[:], wl_in[:])
            w_r = wp.tile([H, HEADS], F32); nc.sync.dma_start(w_r[:], wr_in[:])
            u_lo = wp.tile([128, H], BF16); nc.sync.dma_start(u_lo[:], ulo_in[:])
            u_hi = wp.tile([128, H], BF16); nc.sync.dma_start(u_hi[:], uhi_in[:])
            b_1p = wp.tile([H, 1], F32); nc.sync.dma_start(b_1p[:], b1p[:])
            w_2 = wp.tile([H, C], F32); nc.sync.dma_start(w_2[:], w2_in[:])
            b_2 = wp.tile([C, 1], F32); nc.sync.dma_start(b_2[:], b2c[:])

            ones1 = wp.tile([1, 128], F32)
            nc.vector.memset(ones1[:], 1.0)
            id64f = wp.tile([64, 64], F32)
            make_identity(nc, id64f[:])
            id128f = wp.tile([128, 128], F32)
            make_identity(nc, id128f[:])
            id128b = wp.tile([128, 128], BF16)
            nc.vector.tensor_copy(id128b[:], id128f[:])
            id40f = wp.tile([40, 40], F32)
            make_identity(nc, id40f[:])

            # deginv: per-partition scalar per block -> SBUF [128, NBLK]
            dgi_sb = bigp.tile([128, NBLK], F32)
            nc.sync.dma_start(dgi_sb[:], dgi_in[:].rearrange("b p one -> p (b one)"))

            # persistent feature planes
            h1T = bigp.tile([H, OWN], F32, tag="hT", bufs=2)  # feat-major planes
            h2T = bigp.tile([H, OWN], F32, tag="hT", bufs=2)
            h3T = bigp.tile([H, OWN], F32, tag="hT", bufs=2)
            neighT = bigp.tile([H, OWN], F32)
            er_all = bigp.tile([128, NBLK, HEADS], BF16)
            og_nm = bigp.tile([128, NBLK, 2 * H * 2], BF16)  # node-major GAT out

            # DRAM tables (AGs move compact rows; repack into 256B-stride tables)
            tab1a = dram.tile([HALFR, D], BF16)
            tab1b = dram.tile([HALFR, D], BF16)
            mine2 = dram.tile([OWN, 64], BF16)
            agc2 = dram.tile([NID, 64], BF16, addr_space="Shared")
            tab2a = dram.tile([HALFR, D], BF16)
            tab2b = dram.tile([HALFR, D], BF16)
            mineg = dram.tile([OWN, 68], BF16)
            agcg = dram.tile([NID, 68], BF16, addr_space="Shared")
            tabga = dram.tile([HALFR, D], BF16)
            tabgb = dram.tile([HALFR, D], BF16)

            # ================= P1: embed =================
            # full table (replicated): tab1[i] = bf16(x[i] @ Wemb + b)
            HCH = NCH_ALL // 2
            for ch in range(NCH_ALL):
                xb = sb.tile([IN, CH], F32, tag="xb")
                nc.sync.dma_start(xb[:], xT[:, ch * CH:(ch + 1) * CH])
                stg = sb.tile([128, 4, D], BF16, tag="stg1")
                pe = pp.tile([128, 4, H], F32, space="PSUM", tag="psA", bufs=4)
                for q in range(4):
                    nc.tensor.matmul(pe[:, q, :], xb[:, q * 128:(q + 1) * 128],
                                     w_emb[:], start=True, stop=False)
                    nc.tensor.matmul(pe[:, q, :], ones1[0:1, 0:128], b_embr[0:1, :],
                                     start=False, stop=True)
                nc.vector.tensor_copy(stg[:, :, 0:H], pe[:])
                tdst = tab1a if ch < HCH else tab1b
                roff = ch * CH if ch < HCH else (ch - HCH) * CH
                nc.scalar.dma_start(tdst[roff:roff + CH, :].rearrange(
                    "(p q) d -> p q d", q=4), stg[:])
            # own features, feat-major (f32)
            for ch in range(NCH_OWN):
                xb2 = sb.tile([IN, CH], F32, tag="xb")
                nc.sync.dma_start(xb2[:], xo[:, ch * CH:(ch + 1) * CH])
                ph = pp.tile([H, CH], F32, space="PSUM", tag="psB", bufs=3)
                nc.tensor.matmul(ph[:], w_emb[:], xb2[:], start=True, stop=True)
                nc.scalar.activation(h1T[:, ch * CH:(ch + 1) * CH], ph[:],
                                     mybir.ActivationFunctionType.Identity,
                                     bias=b_embc[:], scale=1.0)

            # ============== SAGE layer helper ==============
            def preload(n, with_mt=False):
                pre = []
                for b in range(n):
                    pit = sb.tile([128, 2, S16], I16, tag="pit", bufs=4)
                    nc.sync.dma_start(pit[:], idx_in[b])
                    pmk = sb.tile([128, TPB * 128], FP8, tag="pmk", bufs=3)
                    nc.sync.dma_start(pmk[:], mask_in[b])
                    pmt = None
                    if with_mt:
                        pmt = sb.tile([128, TPB * 128], FP8, tag="pmt", bufs=3)
                        nc.scalar.dma_start(pmt[:], maskT_in[b])
                    pre.append((pit, pmk, pmt))
                return pre

            def sage_agg(table, pre=()):
                """Aggregate neighbor means into neighT (feat-major, f32)."""
                if isinstance(table, tuple):
                    ta, tb_ = table[0][:], table[1][:]
                else:
                    ta, tb_ = table[0:HALFR, :], table[HALFR:NID, :]
                it4 = None
                for b in range(NBLK):
                    if b % 4 == 0:
                        it4 = sb.tile([128, 4, 2, S16], I16, tag="it", bufs=2)
                        nc.sync.dma_start(it4[:], idx_in[b:b + 4].rearrange(
                            "q p h s -> p q h s"))
                    it = it4
                    mk = sb.tile([128, TPB * 128], FP8, tag="mk", bufs=4)
                    nc.sync.dma_start(mk[:], mask_in[b])
                    g = sb.tile([128, TPB, D], BF16, tag="g", bufs=5)
                    nc.gpsimd.dma_gather(g[:, 0:TPH, :], ta,
                                         it[:, b % 4, 0, :], SLOTH, SLOTH, D)
                    nc.gpsimd.dma_gather(g[:, TPH:TPB, :], tb_,
                                         it[:, b % 4, 1, :], SLOTH, SLOTH, D)
                    pa = pp.tile([128, H], F32, space="PSUM", tag="psA", bufs=4)
                    for t in range(TPB):
                        nc.tensor.matmul(pa[:], mk[:, t * 128:(t + 1) * 128],
                                         g[:, t, 0:H],
                                         start=(t == 0), stop=(t == TPB - 1))
                    nb = sb.tile([128, H], F32, tag="nb")
                    nc.vector.tensor_scalar_mul(nb[:], pa[:], dgi_sb[:, b:b + 1])
                    pt = pp.tile([H, 128], F32, space="PSUM", tag="psA", bufs=4)
                    nc.tensor.transpose(pt[:], nb[:], id128f[:])
                    nc.vector.tensor_copy(neighT[:, b * 128:(b + 1) * 128], pt[:])

            def sage_dense(hT_in, w_s, w_n, b_n, hT_out):
                for ch in range(NCH_OWN):
                    pd = pp.tile([H, CH], F32, space="PSUM", tag="psB", bufs=3)
                    nc.tensor.matmul(pd[:], w_s[:], hT_in[:, ch * CH:(ch + 1) * CH],
                                     start=True, stop=False)
                    nc.tensor.matmul(pd[:], w_n[:], neighT[:, ch * CH:(ch + 1) * CH],
                                     start=False, stop=True)
                    nc.scalar.activation(hT_out[:, ch * CH:(ch + 1) * CH], pd[:],
                                         mybir.ActivationFunctionType.Relu,
                                         bias=b_n[:], scale=1.0)

            def write_rows(hT_src, mine, width, with_el=False):
                """Transpose own feat-major features into node-major bf16 rows."""
                for b in range(NBLK):
                    ptr = pp.tile([128, H], F32, space="PSUM", tag="psA", bufs=4)
                    nc.tensor.transpose(ptr[:], hT_src[:, b * 128:(b + 1) * 128],
                                        id64f[:])
                    stg = sb.tile([128, width], BF16, tag="stg2")
                    nc.vector.tensor_copy(stg[:, 0:H], ptr[:])
                    if with_el:
                        pel = pp.tile([128, HEADS], F32, space="PSUM", tag="psA", bufs=4)
                        nc.tensor.matmul(pel[:], hT_src[:, b * 128:(b + 1) * 128],
                                         w_l[:], start=True, stop=True)
                        nc.vector.tensor_copy(stg[:, H:H + HEADS], pel[:])
                        per = pp.tile([128, HEADS], F32, space="PSUM", tag="psA", bufs=4)
                        nc.tensor.matmul(per[:], hT_src[:, b * 128:(b + 1) * 128],
                                         w_r[:], start=True, stop=True)
                        nc.vector.tensor_copy(er_all[:, b, :], per[:])
                    nc.sync.dma_start(mine[b * 128:(b + 1) * 128, :], stg[:])

            # ================= SAGE 1 =================
            if upto >= 2:
                sage_agg((tab1a, tab1b))
            if upto >= 3:
                sage_dense(h1T, w_s1, w_n1, b_n1, h2T)
                write_rows(h2T, mine2, 64, with_el=False)
                pre2 = []
                nc.gpsimd.collective_compute(
                    "AllGather", mybir.AluOpType.bypass,
                    replica_groups=[list(range(NCORES))],
                    ins=[mine2.opt()], outs=[agc2.opt()],
                )
                nc.sync.dma_start(tab2a[:, 0:64], agc2[0:HALFR, :])
                nc.scalar.dma_start(tab2b[:, 0:64], agc2[HALFR:NID, :])

            preg = []
            # ================= SAGE 2 =================
            if upto >= 4:
                sage_agg((tab2a, tab2b), pre2)
                sage_dense(h2T, w_s2, w_n2, b_n2, h3T)
                write_rows(h3T, mineg, 68, with_el=True)
                preg = []
                nc.gpsimd.collective_compute(
                    "AllGather", mybir.AluOpType.bypass,
                    replica_groups=[list(range(NCORES))],
                    ins=[mineg.opt()], outs=[agcg.opt()],
                )
                nc.sync.dma_start(tabga[:, 0:68], agcg[0:HALFR, :])
                nc.scalar.dma_start(tabgb[:, 0:68], agcg[HALFR:NID, :])

            # ================= GAT aggregation =================
            _noW = _noER = _noPG = _noTR = False
            it4g = None
            for b in range(NBLK if upto >= 5 else 0):
                if b % 4 == 0:
                    it4g = sb.tile([128, 4, 2, S16], I16, tag="it", bufs=2)
                    nc.sync.dma_start(it4g[:], idx_in[b:b + 4].rearrange(
                        "q p h s -> p q h s"))
                mk = sb.tile([128, TPB * 128], FP8, tag="mk", bufs=4)
                nc.sync.dma_start(mk[:], mask_in[b])
                mt = sb.tile([128, TPB * 128], FP8, tag="mt", bufs=3)
                nc.sync.dma_start(mt[:], maskT_in[b])
                g = sb.tile([128, TPB, D], BF16, tag="g", bufs=5)
                nc.gpsimd.dma_gather(g[:, 0:TPH, :], tabga[:],
                                     it4g[:, b % 4, 0, :], SLOTH, SLOTH, D)
                nc.gpsimd.dma_gather(g[:, TPH:TPB, :], tabgb[:],
                                     it4g[:, b % 4, 1, :], SLOTH, SLOTH, D)
                # er broadcast to edge slots via maskT matmuls
                perb = pp.tile([128, TPB, HEADS], F32, space="PSUM", tag="psA", bufs=4)
                for t in range(TPB):
                    nc.tensor.matmul(perb[:, t, :], mt[:, t * 128:(t + 1) * 128],
                                     er_all[:, b, :], start=True, stop=True)
                # e = leaky_relu(el + er); ex = exp(e)  (no Lrelu table: max(x, .2x))
                ee = sb.tile([128, TPB, HEADS], BF16, tag="ee")
                nc.vector.tensor_add(ee[:], g[:, :, H:H + HEADS], perb[:])
                et = sb.tile([128, TPB, HEADS], BF16, tag="et")
                nc.vector.tensor_scalar_mul(et[:], ee[:], SLOPE)
                nc.vector.tensor_max(ee[:], ee[:], et[:])
                wst = sb.tile([128, TPB, HEADS * H + HEADS], BF16, tag="wst", bufs=3)
                nc.scalar.activation(wst[:, :, HEADS * H:], ee[:],
                                     mybir.ActivationFunctionType.Exp)
                pg = pp.tile([128, HEADS * H + HEADS], F32, space="PSUM", tag="psB", bufs=3)
                HT = TPB // 2
                for half in range(2):
                    tsl = slice(half * HT, (half + 1) * HT)
                    if _noW:
                        if b == 0 and half == 0:
                            nc.vector.memset(wst[:, :, 0:HEADS * H], 0.5)
                    else:
                        # fused per-head weighting: wst[p,t,h,f] = g[p,t,f]*ex[p,t,h]
                        nc.vector.tensor_mul(
                            wst[:, tsl, 0:HEADS * H].rearrange(
                                "p t (h f) -> p t h f", h=HEADS),
                            g[:, tsl, 0:H].rearrange(
                                "p t (o f) -> p t o f", o=1).to_broadcast(
                                    [128, HT, HEADS, H]),
                            wst[:, tsl, HEADS * H:].rearrange(
                                "p t (h o) -> p t h o", o=1).to_broadcast(
                                    [128, HT, HEADS, H]))
                    for t in range(half * HT, (half + 1) * HT):
                        nc.tensor.matmul(pg[:], mk[:, t * 128:(t + 1) * 128],
                                         wst[:, t, :], start=(t == 0),
                                         stop=(t == TPB - 1))
                # normalize by z and transpose for the dense phase
                zt = sb.tile([128, HEADS], F32, tag="zt")
                nc.vector.tensor_scalar_max(zt[:], pg[:, HEADS * H:], 1e-20)
                zi = sb.tile([128, HEADS], F32, tag="zi")
                nc.vector.reciprocal(zi[:], zt[:])
                nc.vector.tensor_mul(
                    og_nm[:, b, :].rearrange("p (h f) -> p h f", h=HEADS),
                    pg[:, 0:HEADS * H].rearrange("p (h f) -> p h f", h=HEADS),
                    zi[:].to_broadcast([128, HEADS, H]))

            # ================= GAT dense + classifier =================
            def og_stage(ch, half):
                stgT = sb.tile([128, CH], BF16, tag=f"ogs{half}", bufs=2)
                for q in range(4):
                    b = ch * 4 + q
                    ptg = pp.tile([128, 128], BF16, space="PSUM", tag="psA", bufs=4)
                    nc.tensor.transpose(
                        ptg[:], og_nm[:, b, half * 128:(half + 1) * 128], id128b[:])
                    nc.vector.tensor_copy(stgT[:, q * 128:(q + 1) * 128], ptg[:])
                return stgT

            if upto < 6:
                zo = sb.tile([128, NBLK, C], F32, tag="zo")
                nc.vector.memset(zo[:], 0.0)
                nc.sync.dma_start(
                    out[:].rearrange("(q p) c -> p q c", p=128), zo[:])
            for ch in range(NCH_OWN if upto >= 6 else 0):
                og_loS = og_stage(ch, 0)
                og_hiS = og_stage(ch, 1)
                p4 = pp.tile([H, CH], F32, space="PSUM", tag="psB", bufs=3)
                nc.tensor.matmul(p4[:], u_lo[:], og_loS[:],
                                 start=True, stop=False)
                nc.tensor.matmul(p4[:], u_hi[:], og_hiS[:],
                                 start=False, stop=True)
                h4 = sb.tile([H, CH], F32, tag="h4")
                nc.scalar.activation(h4[:], p4[:],
                                     mybir.ActivationFunctionType.Relu,
                                     bias=b_1p[:], scale=1.0)
                plg = pp.tile([C, CH], F32, space="PSUM", tag="psB", bufs=3)
                nc.tensor.matmul(plg[:], w_2[:], h4[:], start=True, stop=True)
                lg = sb.tile([C, CH], F32, tag="lg")
                nc.scalar.activation(lg[:], plg[:],
                                     mybir.ActivationFunctionType.Identity,
                                     bias=b_2[:], scale=1.0)
                ostg = sb.tile([128, 4, C], F32, tag="ostg")
                for q in range(4):
                    plt = pp.tile([128, C], F32, space="PSUM", tag="psA", bufs=4)
                    nc.tensor.transpose(plt[:], lg[:, q * 128:(q + 1) * 128], id40f[:])
                    nc.vector.tensor_copy(ostg[:, q, :], plt[:])
                nc.sync.dma_start(
                    out[ch * CH:(ch + 1) * CH, :].rearrange("(q p) c -> p q c", p=128),
                    ostg[:])

    nc.compile()
    return nc


def _plan(src, dst):
    """Host-side graph partitioning. Returns per-core index/mask arrays."""
    src = np.asarray(src).astype(np.int64)
    dst = np.asarray(dst).astype(np.int64)
    for seed in range(64):
        rng = np.random.default_rng(seed)
        perm = rng.permutation(NID)[:N].astype(np.int64)  # orig -> internal
        si = perm[src]
        di = perm[dst]
        gblk = di // PB                      # 0..447
        half = (si >= HALFR).astype(np.int64)
        grp = gblk * 2 + half
        cnt = np.bincount(grp, minlength=NCORES * NBLK * 2)
        if cnt.max() <= SLOTH:
            break
    else:
        raise RuntimeError("could not pack edges into halves; increase NBLK")

    order = np.lexsort((si, grp))
    g_sorted = grp[order]
    # position within group
    starts = np.zeros(NCORES * NBLK * 2 + 1, np.int64)
    np.cumsum(cnt, out=starts[1:])
    j_in_grp = np.arange(E, dtype=np.int64) - starts[g_sorted]

    e_si = si[order]
    e_di = di[order]
    e_half = half[order]
    e_gblk = gblk[order]
    e_core = e_gblk // NBLK
    e_blk = e_gblk % NBLK

    # idx arrays [NCORES, NBLK, 16, 2, S16] then replicated to 128 partitions
    idx16 = np.zeros((NCORES, NBLK, 16, 2, S16), np.int16)
    val = np.where(e_half == 0, e_si, e_si - HALFR).astype(np.int16)
    idx16[e_core, e_blk, j_in_grp % 16, e_half, j_in_grp // 16] = val
    idx16 = np.broadcast_to(idx16[:, :, None, :, :, :],
                            (NCORES, NBLK, 8, 16, 2, S16)).reshape(
                                NCORES, NBLK, 128, 2, S16).copy()

    # masks [NCORES, NBLK, 128, TPB*128] fp8: slot (t, p) -> dst col d
    t_of = (e_half * TPH + j_in_grp // 128).astype(np.int64)
    p_of = (j_in_grp % 128).astype(np.int64)
    d_of = (e_di % PB).astype(np.int64)
    m8 = np.zeros((NCORES, NBLK, 128, TPB * 128), np.uint8)
    one_fp8 = np.array(1.0, NP_FP8).view(np.uint8).item()
    m8[e_core, e_blk, p_of, t_of * 128 + d_of] = one_fp8
    mT8 = m8.reshape(NCORES, NBLK, 128, TPB, 128).transpose(0, 1, 4, 3, 2)
    mT8 = np.ascontiguousarray(mT8).reshape(NCORES, NBLK, 128, TPB * 128)

    # deginv per dst slot
    deg = np.bincount(di, minlength=NID).astype(np.float32)
    dgi = (1.0 / np.maximum(deg, 1.0)).reshape(NCORES, NBLK, PB, 1)

    return perm, idx16, m8.view(NP_FP8), mT8.view(NP_FP8), dgi


def kernel(x, src, dst, W_embed, b_embed, Ws1, Wn1, bn1, Ws2, Wn2, bn2,
           Wg, al, ar, bg, W1, b1, W2, b2):
    x = np.asarray(x, np.float32)
    perm, idx16, m8, mT8, dgi = _plan(src, dst)

    if "nc" not in _cached:
        _cached["nc"] = _build_bass()
    nc = _cached["nc"]

    # weight preprocessing
    Wg = np.asarray(Wg, np.float32)
    al = np.asarray(al, np.float32)
    ar = np.asarray(ar, np.float32)
    W1 = np.asarray(W1, np.float32)
    WL = np.stack([Wg[:, h * H:(h + 1) * H] @ al[h] for h in range(HEADS)], 1)
    WR = np.stack([Wg[:, h * H:(h + 1) * H] @ ar[h] for h in range(HEADS)], 1)
    b1p = (np.asarray(b1, np.float32) + np.asarray(bg, np.float32) @ W1)
    U = [Wg[:, h * H:(h + 1) * H] @ W1[h * H:(h + 1) * H] for h in range(HEADS)]
    Ulo = np.vstack([U[0], U[1]]).astype(NP_BF16)
    Uhi = np.vstack([U[2], U[3]]).astype(NP_BF16)

    xT = np.zeros((IN, NID), np.float32)
    xT[:, perm] = x.T
    # embed writes table rows p-major (row = p*4+q within each 512-chunk) so
    # the DMA emits 1KB descriptors; present xT columns in matching order
    xTs = np.ascontiguousarray(
        np.swapaxes(xT.reshape(IN, NCH_ALL, 128, 4), 2, 3).reshape(IN, NID))

    common = {
        "xT": xTs,
        "wemb": np.asarray(W_embed, np.float32),
        "bembr": np.asarray(b_embed, np.float32).reshape(1, H),
        "bembc": np.asarray(b_embed, np.float32).reshape(H, 1),
        "ws1": np.asarray(Ws1, np.float32), "wn1": np.asarray(Wn1, np.float32),
        "bn1": np.asarray(bn1, np.float32).reshape(H, 1),
        "ws2": np.asarray(Ws2, np.float32), "wn2": np.asarray(Wn2, np.float32),
        "bn2": np.asarray(bn2, np.float32).reshape(H, 1),
        "wl_in": WL, "wr_in": WR,
        "ulo_in": Ulo, "uhi_in": Uhi,
        "b1p": b1p.reshape(H, 1),
        "w2_in": np.asarray(W2, np.float32),
        "b2c": np.asarray(b2, np.float32).reshape(C, 1),
    }
    in_maps = []
    for c in range(NCORES):
        m = dict(common)
        m["xo"] = np.ascontiguousarray(xT[:, c * OWN:(c + 1) * OWN])
        m["idx_in"] = np.ascontiguousarray(idx16[c])
        m["mask_in"] = np.ascontiguousarray(m8[c])
        m["maskT_in"] = np.ascontiguousarray(mT8[c])
        m["dgi_in"] = np.ascontiguousarray(dgi[c])
        in_maps.append(m)

    res = run_bass_kernel_spmd(nc, in_maps, core_ids=list(range(NCORES)))
    full = np.concatenate([res.results[c]["out"] for c in range(NCORES)], 0)
    return full[perm].astype(np.float32)

